# revision 7
# baseline (speedup 1.0000x reference)
"""BinaryLinear TRN2 kernel: z = x @ sign(weight).T + bias.

x [8192, 4096] f32, weight [4096, 4096] f32, bias [4096] f32 (zeros).

Strategy (8 NeuronCores, SPMD, no collectives):
  - Data-parallel over the 8192-token batch dim: core c computes rows
    c*1024..(c+1)*1024 of z. weight is replicated to every core.
  - The host passes x^T (f16) and weight^T (f32) - layout/marshaling
    resharding only; sign() and every matmul stay on device. The device
    does ZERO PE transposes: the PE runs nothing but the 2048 N=512
    accumulation matmuls per core (the ~437us roofline at 1 cycle/row,
    measured 443us for the bare stream).
  - All matmul operands are float16 (x rounded to f16: ~2e-4 rel err;
    PSUM accumulation stays f32). Weights are binarized straight to
    +-1 f16 by a single ScalarE Sign activation pass per tile.
  - x^T loads straight into its resident SBUF tile with 2 big DMAs
    issued from the GPSIMD (SWDGE) queue: no staging pool, no cast
    pass, and nothing queues behind them on the ScalarE ring (16
    staged x loads + casts previously cost ~250us/iter of pipeline
    restart through the xt write-after-read barrier).
  - Weights stream HBM->SBUF in 1 MiB DMAs (4 k-tiles x 512 features)
    on the SP HWDGE ring, double-buffered in 512-feature spans. Per
    span the PE has 54.6us of matmuls vs ~25us of weight DMA.
  - Engine roles are exclusive to avoid strict-FIFO head-of-line
    blocking: ScalarE = Sign stream only, VectorE = PSUM->SBUF
    evictions, GPSIMD = x load + z stores, SP = weight stream.
  - Host transposes the per-core z^T shards back on gather.
"""

import numpy as np

import concourse.bacc as bacc
import concourse.bass as bass
import concourse.mybir as mybir
import concourse.tile as tile
from concourse import bass_utils
from concourse.bass import ts

P = 128
N_CORES = 8
N_TOK, K_IN, N_OUT = 8192, 4096, 4096
T = N_TOK // N_CORES  # 1024 tokens per core
KT = K_IN // P  # 32 k-tiles
KQ = 4  # k-tiles per weight DMA / binarize instruction (1 MiB transfers)
SPAN = 512  # output-feature span per weight buffer
NSPAN = N_OUT // SPAN  # 8
NTC = T // 512  # 2 token chunks of 512

F32 = mybir.dt.float32
F16 = mybir.dt.float16

_cached_nc = None


def _build_program(loop: int = 0):
    """loop=0: plain kernel. loop=L>0: body wrapped in an on-device For_i
    (used for HW timing via the slope method)."""
    nc = bacc.Bacc("TRN2", target_bir_lowering=False, debug=False)
    # x^T shard [k, tok] (f16) and W^T full [k, out] (f32); z^T [out, tok].
    xs_d = nc.dram_tensor("xs", [K_IN, T], F16, kind="ExternalInput")
    w_d = nc.dram_tensor("w", [K_IN, N_OUT], F32, kind="ExternalInput")
    zs_d = nc.dram_tensor("zs", [N_OUT, T], F32, kind="ExternalOutput")

    import contextlib

    with tile.TileContext(nc) as tc:
        with (
            tc.tile_pool(name="xtp", bufs=1) as xtp,
            tc.tile_pool(name="wbp", bufs=2) as wbp,
            tc.tile_pool(name="wrawp", bufs=4) as wrawp,
            tc.tile_pool(name="ztp", bufs=4) as ztp,
            tc.tile_pool(name="psm", bufs=3, space="PSUM") as psm,
        ):
            # x^T resident f16: [128 kp, 32 ko, 1024 tok] (64 KiB/part)
            xt = xtp.tile([P, KT, T], F16)

            loop_cm = tc.For_i(0, loop, 1) if loop else contextlib.nullcontext()
            with loop_cm:
                # ---- x shard: two direct DMAs into the resident tile
                # (tc0 first so the first span's matmuls start early) ----
                for tcix in range(NTC):
                    src = xs_d.ap()[:, ts(tcix, 512)]
                    nc.gpsimd.dma_start(
                        xt[:, :, ts(tcix, 512)],
                        src.rearrange("(a p) t -> p a t", p=P),
                    )

                # ---- weight span prep: raw W^T columns stream in, one
                # ScalarE Sign pass binarizes f32 -> +-1 f16 ----
                def prep(s):
                    wb = wbp.tile([P, KT, SPAN], F16, name="wb", tag="wb")
                    for kq in range(KT // KQ):
                        wr = wrawp.tile([P, KQ, SPAN], F32, name="wr", tag="wr")
                        src = w_d.ap()[ts(kq, KQ * P), ts(s, SPAN)]
                        nc.sync.dma_start(
                            wr[:], src.rearrange("(a p) o -> p a o", p=P)
                        )
                        nc.scalar.sign(wb[:, ts(kq, KQ), :], wr[:])
                    return wb

                # ---- software-pipelined spans: prep for span s+1 is emitted
                # before the matmuls of span s ----
                wb_cur = prep(0)
                for s in range(NSPAN):
                    wb_next = prep(s + 1) if s + 1 < NSPAN else None
                    for ot in range(SPAN // P):
                        pm = psm.tile([P, NTC, 512], F32, name="pm", tag="pm")
                        for ko in range(KT):
                            for tcix in range(NTC):
                                nc.tensor.matmul(
                                    pm[:, tcix, :],
                                    wb_cur[:, ko, ts(ot, P)],
                                    xt[:, ko, ts(tcix, 512)],
                                    start=(ko == 0),
                                    stop=(ko == KT - 1),
                                )
                        zt = ztp.tile([P, NTC, 512], F32, name="zt", tag="zt")
                        nc.vector.tensor_copy(zt[:], pm[:])
                        nc.gpsimd.dma_start(
                            zs_d.ap()[ts(s * (SPAN // P) + ot, P), :],
                            zt[:].rearrange("p a b -> p (a b)"),
                        )
                    wb_cur = wb_next
    nc.compile()
    return nc


def _get_nc():
    global _cached_nc
    if _cached_nc is None:
        _cached_nc = _build_program()
    return _cached_nc


def _in_maps(x: np.ndarray, weight: np.ndarray):
    xT16 = np.ascontiguousarray(x.T.astype(np.float16))  # [K_IN, N_TOK]
    wT = np.ascontiguousarray(weight.T)  # [K_IN, N_OUT]
    return [
        {"xs": np.ascontiguousarray(xT16[:, c * T : (c + 1) * T]), "w": wT}
        for c in range(N_CORES)
    ]


def kernel(x: np.ndarray, weight: np.ndarray, bias: np.ndarray) -> np.ndarray:
    x = np.ascontiguousarray(np.asarray(x, dtype=np.float32))
    weight = np.ascontiguousarray(np.asarray(weight, dtype=np.float32))
    bias = np.asarray(bias, dtype=np.float32)
    assert x.shape == (N_TOK, K_IN) and weight.shape == (N_OUT, K_IN)

    nc = _get_nc()
    res = bass_utils.run_bass_kernel_spmd(
        nc, _in_maps(x, weight), core_ids=list(range(N_CORES))
    )
    z = np.empty((N_TOK, N_OUT), dtype=np.float32)
    for c in range(N_CORES):
        np.copyto(z[c * T : (c + 1) * T], res.results[c]["zs"].T)
    if np.any(bias):
        z += bias[None, :]
    return z


# ---------------------------------------------------------------------------
# HW timing support (not used by the grading path; test.py calls this).
# The axon PJRT dispatch overhead (~57 ms) swamps a single kernel execution
# and no NTFF profile hook is available here, so we measure the on-device
# time with a For_i-looped variant: slope of wall time vs loop count.
# ---------------------------------------------------------------------------


def _make_runner(nc, n_cores=N_CORES):
    import jax
    from jax.sharding import Mesh, PartitionSpec
    from jax.experimental.shard_map import shard_map
    from concourse import bass2jax

    bass2jax.install_neuronx_cc_hook()
    partition_name = nc.partition_id_tensor.name if nc.partition_id_tensor else None
    in_names, out_names, out_avals, zero_outs = [], [], [], []
    for alloc in nc.m.functions[0].allocations:
        if not isinstance(alloc, mybir.MemoryLocationSet):
            continue
        name = alloc.memorylocations[0].name
        if alloc.kind == "ExternalInput":
            if name != partition_name:
                in_names.append(name)
        elif alloc.kind == "ExternalOutput":
            out_names.append(name)
            out_avals.append(
                jax.core.ShapedArray(tuple(alloc.tensor_shape), mybir.dt.np(alloc.dtype))
            )
            zero_outs.append(
                np.zeros(tuple(alloc.tensor_shape), mybir.dt.np(alloc.dtype))
            )
    n_params, n_outs = len(in_names), len(out_avals)
    all_in_names = list(in_names) + list(out_names)
    if partition_name is not None:
        all_in_names.append(partition_name)

    def _body(*args):
        operands = list(args)
        if partition_name is not None:
            operands.append(bass2jax.partition_id_tensor())
        return tuple(
            bass2jax._bass_exec_p.bind(
                *operands,
                out_avals=tuple(out_avals),
                in_names=tuple(all_in_names),
                out_names=tuple(out_names),
                lowering_input_output_aliases=(),
                sim_require_finite=True,
                sim_require_nnan=True,
                nc=nc,
            )
        )

    donate = tuple(range(n_params, n_params + n_outs))
    devices = jax.devices()[:n_cores]
    mesh = Mesh(np.asarray(devices), ("core",))
    in_specs = (PartitionSpec("core"),) * (n_params + n_outs)
    out_specs = (PartitionSpec("core"),) * n_outs
    jitted = jax.jit(
        shard_map(_body, mesh=mesh, in_specs=in_specs, out_specs=out_specs,
                  check_rep=False),
        donate_argnums=donate,
        keep_unused=True,
    )
    return jitted, in_names, zero_outs


def _time_looped(nc, in_maps, nrep=3):
    import time
    import jax

    jitted, in_names, zero_outs = _make_runner(nc)
    concat_in = [
        np.concatenate([np.asarray(in_maps[c][name]) for c in range(N_CORES)], axis=0)
        for name in in_names
    ]
    ins = [jax.device_put(a) for a in concat_in]
    jax.block_until_ready(ins)
    zo_global = [np.concatenate([z] * N_CORES, axis=0) for z in zero_outs]
    outs = jitted(*ins, *[jax.device_put(z) for z in zo_global])
    jax.block_until_ready(outs)
    times = []
    for _ in range(nrep):
        zo = [jax.device_put(z) for z in zo_global]
        jax.block_until_ready(zo)
        t0 = time.perf_counter()
        outs = jitted(*ins, *zo)
        jax.block_until_ready(outs)
        times.append(time.perf_counter() - t0)
    return min(times)


def measure_hw_time_ns(inputs, L1=1, L2=2049, nrep=3, rounds=2):
    x = np.ascontiguousarray(np.asarray(inputs["x"], dtype=np.float32))
    weight = np.ascontiguousarray(np.asarray(inputs["weight"], dtype=np.float32))
    in_maps = _in_maps(x, weight)
    nc1 = _build_program(loop=L1)
    nc2 = _build_program(loop=L2)
    # The host<->device dispatch path through the tunnel has high and
    # time-varying overhead (tens of ms, heavy tails), so the loop delta
    # must be large enough that on-device time dominates: with L2-L1 =
    # 2048 iterations (~1s device time) the slope repeats to ~1%.
    slopes = []
    for _ in range(rounds):
        t1 = _time_looped(nc1, in_maps, nrep=nrep)
        t2 = _time_looped(nc2, in_maps, nrep=nrep)
        slopes.append((t2 - t1) / (L2 - L1))
    return min(slopes) * 1e9
